# revision 7
# baseline (speedup 1.0000x reference)
"""Average-Precision (histogram binning) kernel for 8 Trainium2 NeuronCores.

Reference semantics (C=2 classes, T=10 thresholds):
  s = y_pred[:, 1, ...] flattened   (pos_idx is always class 1 when C==2)
  y = y_true flattened
  per threshold t: tp = #(y==1 & s>t), fp = #(y==0 & s>t), fn = #(y==1) - tp
  AP = trapezoid area over (recall, precision) with endpoint padding.

Device strategy (data-parallel over the 12.58M voxels, 1/8 per core):
  v = (1 - 2y) * fp16(exp(k*s))  with k = 11.0
  Counting at f32 thresholds theta*_t = exp(k*t) (nudged off the fp16 grid):
    fp[t] = #(v >  theta*_t)      (label-0 elements)
    tp[t] = #(v < -theta*_t)      (label-1 elements)
    P     = #(v < -0.5)           (label-1 count)
  Each count is a single DVE/ACT pass with a fused free-dim accumulator.
  fp16 quantization only perturbs the *effective* threshold (same perturbed
  threshold for tp and fp), so the P-R points stay on the true P-R curve and
  the AP error is second-order (~1e-5, validated off-line).
  Per-core [1, NB] counts are partition-reduced via TensorE, AllReduced
  across the 8 cores, and the AP formula is evaluated on-device.
"""

import numpy as np

# Toolchain paths (the grading environment may not have them on sys.path).
import sys

for _p in ("/opt/trn_rl_repo", "/opt/pypackages"):
    if _p not in sys.path:
        sys.path.append(_p)

NUM_CORES = 8
P = 128
FTOT = 12288  # per-core free size: 8 * 128 * 12288 = 12,582,912 voxels
FT = 2048  # tile free size
K_SCALE = 11.0
EPS = 1e-7
N_ACT_BOUNDARIES = 0  # boundaries offloaded to the scalar engine (tuned later)
DEBUG = False


def _boundaries(thresholds):
    """Return list of (kind, value) with kind 'gt' (fp side) / 'lt' (tp side).

    Count layout in the accumulator row:
      cols 0..9   fp[t]  = #(v >  theta_t)
      cols 10..19 tp[t]  = #(v < -theta_t)
      col  20     P      = #(v < -0.5)
    """
    th = np.asarray(thresholds, np.float64)
    theta = np.exp(K_SCALE * th).astype(np.float32)
    # nudge off the fp16 grid so strict-compare has no ties
    on_grid = theta.astype(np.float16).astype(np.float32) == theta
    theta = np.where(on_grid, theta * np.float32(1.0 + 2.0**-13), theta)
    bounds = [("gt", float(t)) for t in theta]
    bounds += [("lt", -float(t)) for t in theta]
    bounds += [("lt", -0.5)]
    return bounds


def _build(thresholds):
    from concourse import bacc, mybir
    from concourse import tile

    dt = mybir.dt
    Alu = mybir.AluOpType
    AF = mybir.ActivationFunctionType

    bounds = _boundaries(thresholds)
    NB = len(bounds)  # 21

    nc = bacc.Bacc(
        "TRN2", target_bir_lowering=False, debug=False, num_devices=NUM_CORES
    )
    s_ext = nc.dram_tensor("s", [P, FTOT], dt.float32, kind="ExternalInput")
    y_ext = nc.dram_tensor("y", [P, FTOT], dt.int32, kind="ExternalInput")
    out_ext = nc.dram_tensor("out", [1, 1], dt.float32, kind="ExternalOutput")
    if DEBUG:
        dbg_acc = nc.dram_tensor("dbg_acc", [P, NB], dt.float32, kind="ExternalOutput")
        dbg_cnt = nc.dram_tensor("dbg_cnt", [1, NB], dt.float32, kind="ExternalOutput")
        dbg_g = nc.dram_tensor("dbg_g", [1, NB], dt.float32, kind="ExternalOutput")
        dbg_v = nc.dram_tensor("dbg_v", [P, FT], dt.float32, kind="ExternalOutput")

    NT = FTOT // FT

    with tile.TileContext(nc) as tc:
        with (
            tc.tile_pool(name="io", bufs=3) as io_pool,
            tc.tile_pool(name="mid", bufs=3) as mid_pool,
            tc.tile_pool(name="msk", bufs=3) as msk_pool,
            tc.tile_pool(name="acc", bufs=NT + 1) as acc_pool,
            tc.tile_pool(name="fin", bufs=1) as fin_pool,
            tc.tile_pool(name="psum", bufs=1, space="PSUM") as psum_pool,
            tc.tile_pool(name="dram", bufs=1, space="DRAM") as dram_pool,
        ):
            acc_tiles = []
            for j in range(NT):
                s_t = io_pool.tile([P, FT], dt.float32, tag="s", name=f"s_{j}")
                y_t = io_pool.tile([P, FT], dt.int32, tag="y", name=f"y_{j}")
                nc.sync.dma_start(out=s_t[:], in_=s_ext[:, j * FT : (j + 1) * FT])
                nc.sync.dma_start(out=y_t[:], in_=y_ext[:, j * FT : (j + 1) * FT])

                e_t = mid_pool.tile([P, FT], dt.float16, tag="e", name=f"e_{j}")
                nc.scalar.activation(
                    out=e_t[:], in_=s_t[:], func=AF.Exp, scale=K_SCALE
                )
                m_t = mid_pool.tile([P, FT], dt.float16, tag="m", name=f"m_{j}")
                nc.vector.tensor_scalar(
                    out=m_t[:],
                    in0=y_t[:],
                    scalar1=-2.0,
                    scalar2=1.0,
                    op0=Alu.mult,
                    op1=Alu.add,
                )
                v_t = mid_pool.tile([P, FT], dt.float16, tag="v", name=f"v_{j}")
                nc.vector.tensor_tensor(
                    out=v_t[:], in0=e_t[:], in1=m_t[:], op=Alu.mult
                )
                if DEBUG and j == 0:
                    vf = mid_pool.tile([P, FT], dt.float32, name="vf_dbg")
                    nc.vector.tensor_copy(vf[:], v_t[:])
                    nc.sync.dma_start(out=dbg_v[:], in_=vf[:])

                acc_t = acc_pool.tile(
                    [P, NB], dt.float32, tag="acc", name=f"acc_{j}"
                )
                for b, (kind, thr) in enumerate(bounds):
                    scr = msk_pool.tile(
                        [P, FT], dt.float16, tag="scr", name=f"scr_{j}_{b}"
                    )
                    op = Alu.is_gt if kind == "gt" else Alu.is_lt
                    nc.vector.tensor_scalar(
                        out=scr[:],
                        in0=v_t[:],
                        scalar1=thr,
                        scalar2=0.0,
                        op0=op,
                        op1=Alu.add,
                        accum_out=acc_t[:, b : b + 1],
                    )
                acc_tiles.append(acc_t)

            # merge per-tile accumulators -> [P, NB]
            acc_tot = acc_pool.tile([P, NB], dt.float32, tag="acct", name="acc_tot")
            nc.vector.tensor_tensor(
                out=acc_tot[:], in0=acc_tiles[0][:], in1=acc_tiles[1][:], op=Alu.add
            )
            for j in range(2, NT):
                nc.vector.tensor_tensor(
                    out=acc_tot[:], in0=acc_tot[:], in1=acc_tiles[j][:], op=Alu.add
                )

            # partition reduce via TensorE: [1, NB] = ones[128,1]^T @ acc_tot
            ones = fin_pool.tile([P, 1], dt.float32, name="ones")
            nc.vector.memset(ones[:], 1.0)
            ps = psum_pool.tile([1, NB], dt.float32, name="ps")
            nc.tensor.matmul(ps[:], ones[:], acc_tot[:], start=True, stop=True)
            cnt_row = fin_pool.tile([1, NB], dt.float32, name="cnt_row")
            nc.vector.tensor_copy(cnt_row[:], ps[:])

            # AllReduce the tiny count row across the 8 cores
            cc_in = dram_pool.tile([1, NB], dt.float32, name="cc_in")
            cc_out = dram_pool.tile([1, NB], dt.float32, name="cc_out")
            nc.sync.dma_start(out=cc_in[:], in_=cnt_row[:])
            nc.gpsimd.collective_compute(
                "AllReduce",
                Alu.add,
                replica_groups=[list(range(NUM_CORES))],
                ins=[cc_in.opt()],
                outs=[cc_out.opt()],
            )
            g = fin_pool.tile([1, NB], dt.float32, name="g_row")
            nc.sync.dma_start(out=g[:], in_=cc_out[:])
            if DEBUG:
                nc.sync.dma_start(out=dbg_acc[:], in_=acc_tot[:])
                nc.sync.dma_start(out=dbg_cnt[:], in_=cnt_row[:])
                nc.sync.dma_start(out=dbg_g[:], in_=g[:])

            # ---- AP formula on partition 0 ----
            T = 10
            fp_c = g[:, 0:T]
            tp_c = g[:, T : 2 * T]
            P_c = g[:, 2 * T : 2 * T + 1]

            num = fin_pool.tile([1, T], dt.float32, name="num")  # tp + eps
            nc.vector.tensor_scalar(
                out=num[:], in0=tp_c, scalar1=EPS, scalar2=None, op0=Alu.add
            )
            den = fin_pool.tile([1, T], dt.float32, name="den")  # tp+fp+eps
            nc.vector.tensor_tensor(out=den[:], in0=tp_c, in1=fp_c, op=Alu.add)
            nc.vector.tensor_scalar(
                out=den[:], in0=den[:], scalar1=EPS, scalar2=None, op0=Alu.add
            )
            rden = fin_pool.tile([1, T], dt.float32, name="rden")
            nc.vector.reciprocal(out=rden[:], in_=den[:])
            prec = fin_pool.tile([1, T], dt.float32, name="prec")
            nc.vector.tensor_tensor(out=prec[:], in0=num[:], in1=rden[:], op=Alu.mult)

            denr = fin_pool.tile([1, 1], dt.float32, name="denr")  # P + eps
            nc.vector.tensor_scalar(
                out=denr[:], in0=P_c, scalar1=EPS, scalar2=None, op0=Alu.add
            )
            rdenr = fin_pool.tile([1, 1], dt.float32, name="rdenr")
            nc.vector.reciprocal(out=rdenr[:], in_=denr[:])
            rec = fin_pool.tile([1, T], dt.float32, name="rec")
            nc.vector.tensor_scalar(
                out=rec[:], in0=num[:], scalar1=rdenr[:], scalar2=None, op0=Alu.mult
            )

            p_row = fin_pool.tile([1, T + 2], dt.float32, name="p_row")
            r_row = fin_pool.tile([1, T + 2], dt.float32, name="r_row")
            nc.vector.memset(p_row[:], 0.0)
            nc.vector.memset(r_row[:], 0.0)
            nc.vector.tensor_copy(p_row[:, 1 : T + 1], prec[:])
            nc.vector.memset(p_row[:, T + 1 : T + 2], 1.0)
            nc.vector.tensor_copy(r_row[:, 1 : T + 1], rec[:])
            nc.vector.memset(r_row[:, 0:1], 1.0)

            dr = fin_pool.tile([1, T + 1], dt.float32, name="dr")
            nc.vector.tensor_tensor(
                out=dr[:], in0=r_row[:, 1 : T + 2], in1=r_row[:, 0 : T + 1],
                op=Alu.subtract,
            )
            psum_p = fin_pool.tile([1, T + 1], dt.float32, name="psum_p")
            nc.vector.tensor_tensor(
                out=psum_p[:], in0=p_row[:, 1 : T + 2], in1=p_row[:, 0 : T + 1],
                op=Alu.add,
            )
            prod = fin_pool.tile([1, T + 1], dt.float32, name="prod")
            nc.vector.tensor_tensor(
                out=prod[:], in0=dr[:], in1=psum_p[:], op=Alu.mult
            )
            area2 = fin_pool.tile([1, 1], dt.float32, name="area2")
            nc.vector.tensor_reduce(
                out=area2[:], in_=prod[:], axis=mybir.AxisListType.X, op=Alu.add
            )
            # recall decreases in t -> signed area is negative; |.| == -0.5 * sum
            res = fin_pool.tile([1, 1], dt.float32, name="res")
            nc.vector.tensor_scalar(
                out=res[:], in0=area2[:], scalar1=-0.5, scalar2=None, op0=Alu.mult
            )
            nc.sync.dma_start(out=out_ext[:], in_=res[:])

    nc.compile()
    return nc


def _prepare_inputs(y_pred, y_true):
    s = np.ascontiguousarray(np.asarray(y_pred)[:, 1]).reshape(-1)
    s = s.astype(np.float32, copy=False)
    y = np.asarray(y_true).reshape(-1).astype(np.int32, copy=False)
    n = s.size
    assert n == NUM_CORES * P * FTOT, n
    s_sh = s.reshape(NUM_CORES, P, FTOT)
    y_sh = y.reshape(NUM_CORES, P, FTOT)
    return [
        {"s": s_sh[i], "y": y_sh[i]} for i in range(NUM_CORES)
    ]


def _run(y_pred, y_true, thresholds, trace=False):
    from concourse.bass_utils import run_bass_kernel_spmd

    nc = _build(thresholds)
    in_maps = _prepare_inputs(y_pred, y_true)
    res = run_bass_kernel_spmd(
        nc, in_maps, core_ids=list(range(NUM_CORES)), trace=trace
    )
    out = np.asarray(res.results[0]["out"], np.float32).reshape(())
    return out, res


def kernel(y_pred, y_true, thresholds):
    out, _ = _run(y_pred, y_true, thresholds, trace=False)
    return out


# revision 10
# speedup vs baseline: 1.9051x; 1.9051x over previous
"""Average-Precision (histogram binning) kernel for 8 Trainium2 NeuronCores.

Reference semantics (C=2 classes, T=10 thresholds):
  s = y_pred[:, 1, ...] flattened   (pos_idx is always class 1 when C==2)
  y = y_true flattened
  per threshold t: tp = #(y==1 & s>t), fp = #(y==0 & s>t), fn = #(y==1) - tp
  AP = trapezoid area over (recall, precision) with endpoint padding.

Device strategy (data-parallel over the 12.58M voxels, 1/8 per core):
  v = (1 - 2y) * fp16(exp(k*s))  with k = 11.0
  Counting at f32 thresholds theta*_t = exp(k*t) (off the fp16 grid):
    fp[t] = #(v >  theta*_t)   tp[t] = #(v < -theta*_t)   P = #(v < -0.5)
  fp16 quantization only perturbs the *effective* threshold (identically for
  tp and fp), so P-R points stay on the true P-R curve and the AP error is
  second-order (~2e-5, validated on and off hardware).

  21 boundary counts are computed by three parallel engine lanes:
   - TE lane: DVE makes a bf16 0/1 mask (4x mode), TensorE reduces it with a
     one-hot-column stationary so every boundary accumulates into one
     [NB, 512] PSUM bank across all tiles.
   - ACT lane: Sigmoid(+-1e6*(v - theta)) + fused accum_out = direct count.
   - DVE pair lane: mask_b' in {0,4096} (one 2-op tensor_scalar), then
     scalar_tensor_tensor (v>theta_a) + mask_b' with accum_out packs two
     counts into one lane-accumulator; decoded exactly per tile via
     round-to-nearest f32->int32 conversion.
  Per-core counts are partition-reduced on TensorE, AllReduced across the 8
  cores (tiny [1,32] f32), and the AP formula is evaluated on-device.
"""

import sys

import numpy as np

for _p in ("/opt/trn_rl_repo", "/opt/pypackages"):
    if _p not in sys.path:
        sys.path.append(_p)

NUM_CORES = 8
P = 128
FTOT = 12288  # per-core columns: 8 * 128 * 12288 = 12,582,912 voxels
K_SCALE = 11.0
EPS = 1e-7
BIG = 1.0e6
T = 10

# lane assignment config (tunable)
CFG = {
    "FT": 4096,          # tile free size (pairs decode assumes counts <= FT)
    "n_pairs": 3,        # DVE STT pairs (2 boundaries each)
    "n_act": 7,          # ACT sigmoid singles
    "m_on_act": False,   # build m = 1-2y on ACT instead of DVE
    "io_bufs": 2,
    "PAIR_FIELD": 4096.0,
}
DEBUG = False


def _boundaries(thresholds):
    """21 boundaries on v. Returns list of (kind, value) with kind 'gt'/'lt'.

    Layout: 0..9 fp[t] (gt, +theta), 10..19 tp[t] (lt, -theta), 20 P (lt,-0.5)
    """
    th = np.asarray(thresholds, np.float64)
    theta = np.exp(K_SCALE * th).astype(np.float32)
    on_grid = theta.astype(np.float16).astype(np.float32) == theta
    theta = np.where(on_grid, theta * np.float32(1.0 + 2.0**-13), theta)
    bounds = [("gt", float(t)) for t in theta]
    bounds += [("lt", -float(t)) for t in theta]
    bounds += [("lt", -0.5)]
    return bounds


def _build(thresholds):
    from concourse import bacc, mybir
    from concourse import tile

    dt = mybir.dt
    Alu = mybir.AluOpType
    AF = mybir.ActivationFunctionType

    bounds = _boundaries(thresholds)
    NB = len(bounds)  # 21
    FT = CFG["FT"]
    NT = FTOT // FT
    NCH = FT // 512  # psum chunks per tile
    FIELD = CFG["PAIR_FIELD"]

    n_pairs = CFG["n_pairs"]
    n_act = CFG["n_act"]
    n_te = NB - 2 * n_pairs - n_act
    assert n_te >= 0
    # assignment: pairs take boundaries [0 .. 2*n_pairs), ACT next n_act, TE rest
    pair_idx = [(2 * i, 2 * i + 1) for i in range(n_pairs)]
    act_idx = list(range(2 * n_pairs, 2 * n_pairs + n_act))
    te_idx = list(range(2 * n_pairs + n_act, NB))

    nc = bacc.Bacc(
        "TRN2", target_bir_lowering=False, debug=False, num_devices=NUM_CORES
    )
    s_ext = nc.dram_tensor("s", [P, FTOT], dt.float32, kind="ExternalInput")
    y_ext = nc.dram_tensor("y", [P, FTOT], dt.int32, kind="ExternalInput")
    # host-provided constants: one-hot columns for TE lane + identity
    oh_ext = nc.dram_tensor(
        "oh", [P, max(1, len(te_idx)) * 32], dt.bfloat16, kind="ExternalInput"
    )
    id_ext = nc.dram_tensor("ident", [32, 32], dt.float32, kind="ExternalInput")
    out_ext = nc.dram_tensor("out", [1, 1], dt.float32, kind="ExternalOutput")
    cnt_ext = nc.dram_tensor("counts", [1, 32], dt.float32, kind="ExternalOutput")

    with tile.TileContext(nc) as tc:
        with (
            tc.tile_pool(name="io", bufs=CFG["io_bufs"]) as io_pool,
            tc.tile_pool(name="mid", bufs=2) as mid_pool,
            tc.tile_pool(name="msk", bufs=2) as msk_pool,
            tc.tile_pool(name="acc", bufs=2) as acc_pool,
            tc.tile_pool(name="fin", bufs=1) as fin_pool,
            tc.tile_pool(name="cst", bufs=1) as cst_pool,
            tc.tile_pool(name="psum", bufs=1, space="PSUM") as psum_pool,
            tc.tile_pool(name="psum2", bufs=1, space="PSUM") as psum2_pool,
            tc.tile_pool(name="dram", bufs=1, space="DRAM") as dram_pool,
        ):
            # ---- constants ----
            onescol = cst_pool.tile([P, 1], dt.float32, name="onescol")
            nc.vector.memset(onescol[:], 1.0)
            one_b = cst_pool.tile([P, 1], dt.float32, name="one_b")
            nc.vector.memset(one_b[:], 1.0)
            oh_t = cst_pool.tile([P, max(1, len(te_idx)) * 32], dt.bfloat16,
                                 name="oh_t")
            nc.sync.dma_start(out=oh_t[:], in_=oh_ext[:])
            ident = cst_pool.tile([32, 32], dt.float32, name="ident")
            nc.sync.dma_start(out=ident[:], in_=id_ext[:])
            act_bias = []
            for i, b in enumerate(act_idx):
                kind, thr = bounds[b]
                bias = cst_pool.tile([P, 1], dt.float32, name=f"abias_{i}")
                # gt: sigmoid(BIG*v - BIG*thr); lt: sigmoid(-BIG*v + BIG*thr)
                nc.vector.memset(bias[:], -BIG * thr if kind == "gt" else BIG * thr)
                act_bias.append(bias)

            # running accumulators
            np_cols = max(1, n_pairs)
            acc_lo = fin_pool.tile([P, np_cols], dt.float32, name="acc_lo")
            acc_hi = fin_pool.tile([P, np_cols], dt.int32, name="acc_hi")
            nc.vector.memset(acc_lo[:], 0.0)
            nc.vector.memset(acc_hi[:], 0)
            acc_act = fin_pool.tile([P, max(1, n_act)], dt.float32, name="acc_act")
            nc.vector.memset(acc_act[:], 0.0)

            ps_te = psum_pool.tile([max(1, len(te_idx)), 512], dt.float32,
                                   name="ps_te")
            first_mm = [True]

            for j in range(NT):
                s_t = io_pool.tile([P, FT], dt.float32, tag="s", name=f"s_{j}")
                y_t = io_pool.tile([P, FT], dt.int32, tag="y", name=f"y_{j}")
                nc.sync.dma_start(out=s_t[:], in_=s_ext[:, j * FT:(j + 1) * FT])
                nc.sync.dma_start(out=y_t[:], in_=y_ext[:, j * FT:(j + 1) * FT])

                e_t = mid_pool.tile([P, FT], dt.float16, tag="e", name=f"e_{j}")
                nc.scalar.activation(out=e_t[:], in_=s_t[:], func=AF.Exp,
                                     scale=K_SCALE)
                m_t = mid_pool.tile([P, FT], dt.float16, tag="m", name=f"m_{j}")
                if CFG["m_on_act"]:
                    nc.scalar.activation(out=m_t[:], in_=y_t[:], func=AF.Copy,
                                         bias=1.0, scale=-2.0)
                else:
                    nc.vector.tensor_scalar(out=m_t[:], in0=y_t[:], scalar1=-2.0,
                                            scalar2=1.0, op0=Alu.mult, op1=Alu.add)
                v_t = mid_pool.tile([P, FT], dt.float16, tag="v", name=f"v_{j}")
                nc.vector.tensor_tensor(out=v_t[:], in0=e_t[:], in1=m_t[:],
                                        op=Alu.mult)
                if DEBUG and j == 0:
                    vf = mid_pool.tile([P, FT], dt.float32, name="vf_dbg")
                    nc.vector.tensor_copy(vf[:], v_t[:])
                    nc.sync.dma_start(out=dbg_v[:], in_=vf[:])

                # ---- TE lane ----
                for i, b in enumerate(te_idx):
                    kind, thr = bounds[b]
                    op = Alu.is_gt if kind == "gt" else Alu.is_lt
                    mk = msk_pool.tile([P, FT], dt.bfloat16, tag="temask",
                                       name=f"mk_{j}_{i}")
                    nc.vector.tensor_scalar(out=mk[:], in0=v_t[:], scalar1=thr,
                                            scalar2=None, op0=op)
                    for c in range(NCH):
                        nc.tensor.matmul(
                            ps_te[:],
                            oh_t[:, i * 32: i * 32 + len(te_idx)],
                            mk[:, c * 512:(c + 1) * 512],
                            start=first_mm[0],
                            stop=(j == NT - 1 and i == len(te_idx) - 1
                                  and c == NCH - 1),
                        )
                        first_mm[0] = False

                # ---- ACT lane ----
                acc_act_t = acc_pool.tile([P, max(1, n_act)], dt.float32,
                                          tag="aat", name=f"aat_{j}")
                for i, b in enumerate(act_idx):
                    kind, thr = bounds[b]
                    scl = BIG if kind == "gt" else -BIG
                    sg = msk_pool.tile([P, FT], dt.float16, tag="sgm",
                                       name=f"sg_{j}_{i}")
                    nc.scalar.activation(out=sg[:], in_=v_t[:], func=AF.Sigmoid,
                                         bias=act_bias[i][:], scale=scl,
                                         accum_out=acc_act_t[:, i:i + 1])
                if n_act:
                    nc.vector.tensor_tensor(out=acc_act[:], in0=acc_act[:],
                                            in1=acc_act_t[:], op=Alu.add)

                # ---- DVE pair lane ----
                acc_pair_t = acc_pool.tile([P, np_cols], dt.float32, tag="apt",
                                           name=f"apt_{j}")
                for i, (ba, bb) in enumerate(pair_idx):
                    ka, tha = bounds[ba]
                    kb, thb = bounds[bb]
                    opa = Alu.is_gt if ka == "gt" else Alu.is_lt
                    opb = Alu.is_gt if kb == "gt" else Alu.is_lt
                    mp = msk_pool.tile([P, FT], dt.float16, tag="pm",
                                       name=f"pm_{j}_{i}")
                    nc.vector.tensor_scalar(out=mp[:], in0=v_t[:], scalar1=thb,
                                            scalar2=FIELD, op0=opb, op1=Alu.mult)
                    po = msk_pool.tile([P, FT], dt.float16, tag="po",
                                       name=f"po_{j}_{i}")
                    nc.vector.scalar_tensor_tensor(
                        out=po[:], in0=v_t[:], scalar=tha, in1=mp[:],
                        op0=opa, op1=Alu.add,
                        accum_out=acc_pair_t[:, i:i + 1],
                    )
                if n_pairs:
                    # decode: hi = rne_int32(a/FIELD - 0.49988); lo = a - FIELD*hi
                    hi_t = acc_pool.tile([P, np_cols], dt.int32, tag="hit",
                                         name=f"hi_{j}")
                    nc.vector.tensor_scalar(
                        out=hi_t[:], in0=acc_pair_t[:], scalar1=1.0 / FIELD,
                        scalar2=0.5 - 0.5 / FIELD, op0=Alu.mult,
                        op1=Alu.subtract,
                    )
                    lo_t = acc_pool.tile([P, np_cols], dt.float32, tag="lot",
                                         name=f"lo_{j}")
                    nc.vector.scalar_tensor_tensor(
                        out=lo_t[:], in0=hi_t[:], scalar=-FIELD,
                        in1=acc_pair_t[:], op0=Alu.mult, op1=Alu.add,
                    )
                    nc.vector.tensor_tensor(out=acc_hi[:], in0=acc_hi[:],
                                            in1=hi_t[:], op=Alu.add)
                    nc.vector.tensor_tensor(out=acc_lo[:], in0=acc_lo[:],
                                            in1=lo_t[:], op=Alu.add)

            # ================= finalize =================
            # stack DVE/ACT accumulators -> [P, W] f32 -> partition reduce
            W = 2 * np_cols + max(1, n_act)
            stack = fin_pool.tile([P, W], dt.float32, name="stack")
            acc_hi_f = fin_pool.tile([P, np_cols], dt.float32, name="acc_hi_f")
            nc.vector.tensor_copy(acc_hi_f[:], acc_hi[:])
            nc.vector.tensor_copy(stack[:, 0:np_cols], acc_lo[:])
            nc.vector.tensor_copy(stack[:, np_cols:2 * np_cols], acc_hi_f[:])
            nc.vector.tensor_copy(stack[:, 2 * np_cols:W], acc_act[:])
            ps_row = psum2_pool.tile([1, W], dt.float32, name="ps_row")
            nc.tensor.matmul(ps_row[:], onescol[:], stack[:], start=True,
                             stop=True)

            # TE psum -> row: copy to sbuf, free-reduce, transpose via identity
            nte = max(1, len(te_idx))
            te_sb = fin_pool.tile([nte, 512], dt.float32, name="te_sb")
            nc.vector.tensor_copy(te_sb[:], ps_te[:])
            te_col = fin_pool.tile([nte, 1], dt.float32, name="te_col")
            nc.vector.tensor_reduce(out=te_col[:], in_=te_sb[:],
                                    axis=mybir.AxisListType.X, op=Alu.add)
            ps_ter = psum2_pool.tile([1, nte], dt.float32, name="ps_ter")
            nc.tensor.matmul(ps_ter[:], te_col[:], ident[:nte, :nte],
                             start=True, stop=True)

            # canonical count row [1, NB] (boundary order)
            row = fin_pool.tile([1, 32], dt.float32, name="row")
            nc.vector.memset(row[:], 0.0)
            for i in range(n_pairs):  # pair i covers boundaries 2i (lo), 2i+1 (hi)
                nc.vector.tensor_copy(row[:, 2 * i:2 * i + 1],
                                      ps_row[:, i:i + 1])
                nc.vector.tensor_copy(row[:, 2 * i + 1:2 * i + 2],
                                      ps_row[:, np_cols + i:np_cols + i + 1])
            if n_act:
                nc.vector.tensor_copy(
                    row[:, 2 * n_pairs:2 * n_pairs + n_act],
                    ps_row[:, 2 * np_cols:2 * np_cols + n_act])
            if te_idx:
                nc.vector.tensor_copy(row[:, 2 * n_pairs + n_act:NB],
                                      ps_ter[:, 0:len(te_idx)])

            # AllReduce [1, 32] across cores via DRAM bounce
            cc_in = dram_pool.tile([1, 32], dt.float32, name="cc_in")
            cc_out = dram_pool.tile([1, 32], dt.float32, name="cc_out")
            nc.sync.dma_start(out=cc_in[:], in_=row[:])
            nc.gpsimd.collective_compute(
                "AllReduce", Alu.add,
                replica_groups=[list(range(NUM_CORES))],
                ins=[cc_in.opt()], outs=[cc_out.opt()],
            )
            g = fin_pool.tile([1, 32], dt.float32, name="g_row")
            nc.sync.dma_start(out=g[:], in_=cc_out[:])
            nc.sync.dma_start(out=cnt_ext[:], in_=g[:])

            # ---- AP formula on partition 0 ----
            fp_c = g[:, 0:T]
            tp_c = g[:, T:2 * T]
            P_c = g[:, 2 * T:2 * T + 1]

            num = fin_pool.tile([1, T], dt.float32, name="num")  # tp+eps
            nc.vector.tensor_scalar(out=num[:], in0=tp_c, scalar1=EPS,
                                    scalar2=None, op0=Alu.add)
            den = fin_pool.tile([1, T], dt.float32, name="den")  # tp+fp+eps
            nc.vector.scalar_tensor_tensor(out=den[:], in0=tp_c, scalar=EPS,
                                           in1=fp_c, op0=Alu.add, op1=Alu.add)
            rden = fin_pool.tile([1, T], dt.float32, name="rden")
            nc.vector.reciprocal(out=rden[:], in_=den[:])
            prec = fin_pool.tile([1, T], dt.float32, name="prec")
            nc.vector.tensor_tensor(out=prec[:], in0=num[:], in1=rden[:],
                                    op=Alu.mult)
            denr = fin_pool.tile([1, 1], dt.float32, name="denr")
            nc.vector.tensor_scalar(out=denr[:], in0=P_c, scalar1=EPS,
                                    scalar2=None, op0=Alu.add)
            rdenr = fin_pool.tile([1, 1], dt.float32, name="rdenr")
            nc.vector.reciprocal(out=rdenr[:], in_=denr[:])
            rec = fin_pool.tile([1, T], dt.float32, name="rec")
            nc.vector.tensor_scalar(out=rec[:], in0=num[:], scalar1=rdenr[:],
                                    scalar2=None, op0=Alu.mult)

            p_row = fin_pool.tile([1, T + 2], dt.float32, name="p_row")
            r_row = fin_pool.tile([1, T + 2], dt.float32, name="r_row")
            nc.vector.memset(p_row[:], 0.0)
            nc.vector.memset(r_row[:], 0.0)
            nc.vector.tensor_copy(p_row[:, 1:T + 1], prec[:])
            nc.vector.memset(p_row[:, T + 1:T + 2], 1.0)
            nc.vector.tensor_copy(r_row[:, 1:T + 1], rec[:])
            nc.vector.memset(r_row[:, 0:1], 1.0)

            dr = fin_pool.tile([1, T + 1], dt.float32, name="dr")
            nc.vector.tensor_tensor(out=dr[:], in0=r_row[:, 1:T + 2],
                                    in1=r_row[:, 0:T + 1], op=Alu.subtract)
            ps_p = fin_pool.tile([1, T + 1], dt.float32, name="ps_p")
            nc.vector.tensor_tensor(out=ps_p[:], in0=p_row[:, 1:T + 2],
                                    in1=p_row[:, 0:T + 1], op=Alu.add)
            prod = fin_pool.tile([1, T + 1], dt.float32, name="prod")
            nc.vector.tensor_tensor(out=prod[:], in0=dr[:], in1=ps_p[:],
                                    op=Alu.mult)
            area2 = fin_pool.tile([1, 1], dt.float32, name="area2")
            nc.vector.tensor_reduce(out=area2[:], in_=prod[:],
                                    axis=mybir.AxisListType.X, op=Alu.add)
            # recall decreases in t -> signed area <= 0; |area| = -0.5*sum
            res = fin_pool.tile([1, 1], dt.float32, name="res")
            nc.vector.tensor_scalar(out=res[:], in0=area2[:], scalar1=-0.5,
                                    scalar2=None, op0=Alu.mult)
            nc.sync.dma_start(out=out_ext[:], in_=res[:])

    nc.compile()
    return nc, len(te_idx)


def _host_consts(n_te):
    # one-hot column matrices for the TE lane: oh[:, i*32 + i2] layout —
    # block i is a [128, n_te] matrix whose column i is ones.
    nte = max(1, n_te)
    oh = np.zeros((P, nte * 32), dtype=np.float32)
    for i in range(n_te):
        oh[:, i * 32 + i] = 1.0
    from ml_dtypes import bfloat16

    oh = oh.astype(bfloat16)
    ident = np.eye(32, dtype=np.float32)
    return oh, ident


def _prepare_inputs(y_pred, y_true, n_te):
    s = np.ascontiguousarray(np.asarray(y_pred)[:, 1]).reshape(-1)
    s = s.astype(np.float32, copy=False)
    y = np.asarray(y_true).reshape(-1).astype(np.int32, copy=False)
    n = s.size
    assert n == NUM_CORES * P * FTOT, n
    s_sh = s.reshape(NUM_CORES, P, FTOT)
    y_sh = y.reshape(NUM_CORES, P, FTOT)
    oh, ident = _host_consts(n_te)
    return [
        {"s": s_sh[i], "y": y_sh[i], "oh": oh, "ident": ident}
        for i in range(NUM_CORES)
    ]


def _run(y_pred, y_true, thresholds, trace=False):
    from concourse.bass_utils import run_bass_kernel_spmd

    nc, n_te = _build(thresholds)
    in_maps = _prepare_inputs(y_pred, y_true, n_te)
    res = run_bass_kernel_spmd(
        nc, in_maps, core_ids=list(range(NUM_CORES)), trace=trace
    )
    out = np.asarray(res.results[0]["out"], np.float32).reshape(())
    return out, res


def kernel(y_pred, y_true, thresholds):
    out, _ = _run(y_pred, y_true, thresholds, trace=False)
    return out


# revision 11
# speedup vs baseline: 1.9609x; 1.0293x over previous
"""Average-Precision (histogram binning) kernel for 8 Trainium2 NeuronCores.

Reference semantics (C=2 classes, T=10 thresholds):
  s = y_pred[:, 1, ...] flattened   (pos_idx is always class 1 when C==2)
  y = y_true flattened
  per threshold t: tp = #(y==1 & s>t), fp = #(y==0 & s>t), fn = #(y==1) - tp
  AP = trapezoid area over (recall, precision) with endpoint padding.

Device strategy (data-parallel over the 12.58M voxels, 1/8 per core):
  v = (1 - 2y) * fp16(exp(k*s))  with k = 11.0
  Counting at f32 thresholds theta*_t = exp(k*t) (off the fp16 grid):
    fp[t] = #(v >  theta*_t)   tp[t] = #(v < -theta*_t)   P = #(v < -0.5)
  fp16 quantization only perturbs the *effective* threshold (identically for
  tp and fp), so P-R points stay on the true P-R curve and the AP error is
  second-order (~2e-5, validated on and off hardware).

  21 boundary counts are computed by three parallel engine lanes:
   - TE lane: DVE makes a bf16 0/1 mask (4x mode), TensorE reduces it with a
     one-hot-column stationary so every boundary accumulates into one
     [NB, 512] PSUM bank across all tiles.
   - ACT lane: Sigmoid(+-1e6*(v - theta)) + fused accum_out = direct count.
   - DVE pair lane: mask_b' in {0,4096} (one 2-op tensor_scalar), then
     scalar_tensor_tensor (v>theta_a) + mask_b' with accum_out packs two
     counts into one lane-accumulator; decoded exactly per tile via
     round-to-nearest f32->int32 conversion.
  Per-core counts are partition-reduced on TensorE, AllReduced across the 8
  cores (tiny [1,32] f32), and the AP formula is evaluated on-device.
"""

import sys

import numpy as np

for _p in ("/opt/trn_rl_repo", "/opt/pypackages"):
    if _p not in sys.path:
        sys.path.append(_p)

NUM_CORES = 8
P = 128
FTOT = 12288  # per-core columns: 8 * 128 * 12288 = 12,582,912 voxels
K_SCALE = 11.0
EPS = 1e-7
BIG = 1.0e6
T = 10

# lane assignment config (tunable)
CFG = {
    "FT": 3072,          # tile free size (pairs decode assumes counts <= FT)
    "n_pairs": 3,        # DVE STT pairs (2 boundaries each)
    "n_act": 7,          # ACT sigmoid singles
    "m_on_act": False,   # build m = 1-2y on ACT instead of DVE
    "io_bufs": 3,
    "PAIR_FIELD": 4096.0,
}
DEBUG = False


def _boundaries(thresholds):
    """21 boundaries on v. Returns list of (kind, value) with kind 'gt'/'lt'.

    Layout: 0..9 fp[t] (gt, +theta), 10..19 tp[t] (lt, -theta), 20 P (lt,-0.5)
    """
    th = np.asarray(thresholds, np.float64)
    theta = np.exp(K_SCALE * th).astype(np.float32)
    on_grid = theta.astype(np.float16).astype(np.float32) == theta
    theta = np.where(on_grid, theta * np.float32(1.0 + 2.0**-13), theta)
    bounds = [("gt", float(t)) for t in theta]
    bounds += [("lt", -float(t)) for t in theta]
    bounds += [("lt", -0.5)]
    return bounds


def _build(thresholds):
    from concourse import bacc, mybir
    from concourse import tile

    dt = mybir.dt
    Alu = mybir.AluOpType
    AF = mybir.ActivationFunctionType

    bounds = _boundaries(thresholds)
    NB = len(bounds)  # 21
    FT = CFG["FT"]
    NT = FTOT // FT
    NCH = FT // 512  # psum chunks per tile
    FIELD = CFG["PAIR_FIELD"]

    n_pairs = CFG["n_pairs"]
    n_act = CFG["n_act"]
    n_te = NB - 2 * n_pairs - n_act
    assert n_te >= 0
    # assignment: pairs take boundaries [0 .. 2*n_pairs), ACT next n_act, TE rest
    pair_idx = [(2 * i, 2 * i + 1) for i in range(n_pairs)]
    act_idx = list(range(2 * n_pairs, 2 * n_pairs + n_act))
    te_idx = list(range(2 * n_pairs + n_act, NB))

    nc = bacc.Bacc(
        "TRN2", target_bir_lowering=False, debug=False, num_devices=NUM_CORES
    )
    s_ext = nc.dram_tensor("s", [P, FTOT], dt.float32, kind="ExternalInput")
    y_ext = nc.dram_tensor("y", [P, FTOT], dt.int32, kind="ExternalInput")
    # host-provided constants: one-hot columns for TE lane + identity
    oh_ext = nc.dram_tensor(
        "oh", [P, max(1, len(te_idx)) * 32], dt.bfloat16, kind="ExternalInput"
    )
    id_ext = nc.dram_tensor("ident", [32, 32], dt.float32, kind="ExternalInput")
    out_ext = nc.dram_tensor("out", [1, 1], dt.float32, kind="ExternalOutput")
    cnt_ext = nc.dram_tensor("counts", [1, 32], dt.float32, kind="ExternalOutput")

    with tile.TileContext(nc) as tc:
        with (
            tc.tile_pool(name="io", bufs=CFG["io_bufs"]) as io_pool,
            tc.tile_pool(name="mid", bufs=3) as mid_pool,
            tc.tile_pool(name="msk", bufs=2) as msk_pool,
            tc.tile_pool(name="acc", bufs=2) as acc_pool,
            tc.tile_pool(name="fin", bufs=1) as fin_pool,
            tc.tile_pool(name="cst", bufs=1) as cst_pool,
            tc.tile_pool(name="psum", bufs=1, space="PSUM") as psum_pool,
            tc.tile_pool(name="psum2", bufs=1, space="PSUM") as psum2_pool,
            tc.tile_pool(name="dram", bufs=1, space="DRAM") as dram_pool,
        ):
            # ---- constants ----
            onescol = cst_pool.tile([P, 1], dt.float32, name="onescol")
            nc.vector.memset(onescol[:], 1.0)
            one_b = cst_pool.tile([P, 1], dt.float32, name="one_b")
            nc.vector.memset(one_b[:], 1.0)
            oh_t = cst_pool.tile([P, max(1, len(te_idx)) * 32], dt.bfloat16,
                                 name="oh_t")
            nc.sync.dma_start(out=oh_t[:], in_=oh_ext[:])
            ident = cst_pool.tile([32, 32], dt.float32, name="ident")
            nc.sync.dma_start(out=ident[:], in_=id_ext[:])
            act_bias = []
            for i, b in enumerate(act_idx):
                kind, thr = bounds[b]
                bias = cst_pool.tile([P, 1], dt.float32, name=f"abias_{i}")
                # gt: sigmoid(BIG*v - BIG*thr); lt: sigmoid(-BIG*v + BIG*thr)
                nc.vector.memset(bias[:], -BIG * thr if kind == "gt" else BIG * thr)
                act_bias.append(bias)

            # running accumulators
            np_cols = max(1, n_pairs)
            acc_lo = fin_pool.tile([P, np_cols], dt.float32, name="acc_lo")
            acc_hi = fin_pool.tile([P, np_cols], dt.int32, name="acc_hi")
            nc.vector.memset(acc_lo[:], 0.0)
            nc.vector.memset(acc_hi[:], 0)
            acc_act = fin_pool.tile([P, max(1, n_act)], dt.float32, name="acc_act")
            nc.vector.memset(acc_act[:], 0.0)

            ps_te = psum_pool.tile([max(1, len(te_idx)), 512], dt.float32,
                                   name="ps_te")
            first_mm = [True]

            for j in range(NT):
                s_t = io_pool.tile([P, FT], dt.float32, tag="s", name=f"s_{j}")
                y_t = io_pool.tile([P, FT], dt.int32, tag="y", name=f"y_{j}")
                nc.sync.dma_start(out=s_t[:], in_=s_ext[:, j * FT:(j + 1) * FT])
                nc.sync.dma_start(out=y_t[:], in_=y_ext[:, j * FT:(j + 1) * FT])

                e_t = mid_pool.tile([P, FT], dt.float16, tag="e", name=f"e_{j}")
                nc.scalar.activation(out=e_t[:], in_=s_t[:], func=AF.Exp,
                                     scale=K_SCALE)
                m_t = mid_pool.tile([P, FT], dt.float16, tag="m", name=f"m_{j}")
                if CFG["m_on_act"]:
                    nc.scalar.activation(out=m_t[:], in_=y_t[:], func=AF.Copy,
                                         bias=1.0, scale=-2.0)
                else:
                    nc.vector.tensor_scalar(out=m_t[:], in0=y_t[:], scalar1=-2.0,
                                            scalar2=1.0, op0=Alu.mult, op1=Alu.add)
                v_t = mid_pool.tile([P, FT], dt.float16, tag="v", name=f"v_{j}")
                nc.vector.tensor_tensor(out=v_t[:], in0=e_t[:], in1=m_t[:],
                                        op=Alu.mult)
                if DEBUG and j == 0:
                    vf = mid_pool.tile([P, FT], dt.float32, name="vf_dbg")
                    nc.vector.tensor_copy(vf[:], v_t[:])
                    nc.sync.dma_start(out=dbg_v[:], in_=vf[:])

                # ---- TE lane ----
                for i, b in enumerate(te_idx):
                    kind, thr = bounds[b]
                    op = Alu.is_gt if kind == "gt" else Alu.is_lt
                    mk = msk_pool.tile([P, FT], dt.bfloat16, tag="temask",
                                       name=f"mk_{j}_{i}")
                    nc.vector.tensor_scalar(out=mk[:], in0=v_t[:], scalar1=thr,
                                            scalar2=None, op0=op)
                    for c in range(NCH):
                        nc.tensor.matmul(
                            ps_te[:],
                            oh_t[:, i * 32: i * 32 + len(te_idx)],
                            mk[:, c * 512:(c + 1) * 512],
                            start=first_mm[0],
                            stop=(j == NT - 1 and i == len(te_idx) - 1
                                  and c == NCH - 1),
                        )
                        first_mm[0] = False

                # ---- ACT lane ----
                acc_act_t = acc_pool.tile([P, max(1, n_act)], dt.float32,
                                          tag="aat", name=f"aat_{j}")
                for i, b in enumerate(act_idx):
                    kind, thr = bounds[b]
                    scl = BIG if kind == "gt" else -BIG
                    sg = msk_pool.tile([P, FT], dt.float16, tag="sgm",
                                       name=f"sg_{j}_{i}")
                    nc.scalar.activation(out=sg[:], in_=v_t[:], func=AF.Sigmoid,
                                         bias=act_bias[i][:], scale=scl,
                                         accum_out=acc_act_t[:, i:i + 1])
                if n_act:
                    nc.vector.tensor_tensor(out=acc_act[:], in0=acc_act[:],
                                            in1=acc_act_t[:], op=Alu.add)

                # ---- DVE pair lane ----
                acc_pair_t = acc_pool.tile([P, np_cols], dt.float32, tag="apt",
                                           name=f"apt_{j}")
                for i, (ba, bb) in enumerate(pair_idx):
                    ka, tha = bounds[ba]
                    kb, thb = bounds[bb]
                    opa = Alu.is_gt if ka == "gt" else Alu.is_lt
                    opb = Alu.is_gt if kb == "gt" else Alu.is_lt
                    mp = msk_pool.tile([P, FT], dt.float16, tag="pm",
                                       name=f"pm_{j}_{i}")
                    nc.vector.tensor_scalar(out=mp[:], in0=v_t[:], scalar1=thb,
                                            scalar2=FIELD, op0=opb, op1=Alu.mult)
                    po = msk_pool.tile([P, FT], dt.float16, tag="po",
                                       name=f"po_{j}_{i}")
                    nc.vector.scalar_tensor_tensor(
                        out=po[:], in0=v_t[:], scalar=tha, in1=mp[:],
                        op0=opa, op1=Alu.add,
                        accum_out=acc_pair_t[:, i:i + 1],
                    )
                if n_pairs:
                    # decode: hi = rne_int32(a/FIELD - 0.49988); lo = a - FIELD*hi
                    hi_t = acc_pool.tile([P, np_cols], dt.int32, tag="hit",
                                         name=f"hi_{j}")
                    nc.vector.tensor_scalar(
                        out=hi_t[:], in0=acc_pair_t[:], scalar1=1.0 / FIELD,
                        scalar2=0.5 - 0.5 / FIELD, op0=Alu.mult,
                        op1=Alu.subtract,
                    )
                    lo_t = acc_pool.tile([P, np_cols], dt.float32, tag="lot",
                                         name=f"lo_{j}")
                    nc.vector.scalar_tensor_tensor(
                        out=lo_t[:], in0=hi_t[:], scalar=-FIELD,
                        in1=acc_pair_t[:], op0=Alu.mult, op1=Alu.add,
                    )
                    nc.vector.tensor_tensor(out=acc_hi[:], in0=acc_hi[:],
                                            in1=hi_t[:], op=Alu.add)
                    nc.vector.tensor_tensor(out=acc_lo[:], in0=acc_lo[:],
                                            in1=lo_t[:], op=Alu.add)

            # ================= finalize =================
            # stack DVE/ACT accumulators -> [P, W] f32 -> partition reduce
            W = 2 * np_cols + max(1, n_act)
            stack = fin_pool.tile([P, W], dt.float32, name="stack")
            acc_hi_f = fin_pool.tile([P, np_cols], dt.float32, name="acc_hi_f")
            nc.vector.tensor_copy(acc_hi_f[:], acc_hi[:])
            nc.vector.tensor_copy(stack[:, 0:np_cols], acc_lo[:])
            nc.vector.tensor_copy(stack[:, np_cols:2 * np_cols], acc_hi_f[:])
            nc.vector.tensor_copy(stack[:, 2 * np_cols:W], acc_act[:])
            ps_row = psum2_pool.tile([1, W], dt.float32, name="ps_row")
            nc.tensor.matmul(ps_row[:], onescol[:], stack[:], start=True,
                             stop=True)

            # TE psum -> row: copy to sbuf, free-reduce, transpose via identity
            nte = max(1, len(te_idx))
            te_sb = fin_pool.tile([nte, 512], dt.float32, name="te_sb")
            nc.vector.tensor_copy(te_sb[:], ps_te[:])
            te_col = fin_pool.tile([nte, 1], dt.float32, name="te_col")
            nc.vector.tensor_reduce(out=te_col[:], in_=te_sb[:],
                                    axis=mybir.AxisListType.X, op=Alu.add)
            ps_ter = psum2_pool.tile([1, nte], dt.float32, name="ps_ter")
            nc.tensor.matmul(ps_ter[:], te_col[:], ident[:nte, :nte],
                             start=True, stop=True)

            # canonical count row [1, NB] (boundary order)
            row = fin_pool.tile([1, 32], dt.float32, name="row")
            nc.vector.memset(row[:], 0.0)
            for i in range(n_pairs):  # pair i covers boundaries 2i (lo), 2i+1 (hi)
                nc.vector.tensor_copy(row[:, 2 * i:2 * i + 1],
                                      ps_row[:, i:i + 1])
                nc.vector.tensor_copy(row[:, 2 * i + 1:2 * i + 2],
                                      ps_row[:, np_cols + i:np_cols + i + 1])
            if n_act:
                nc.vector.tensor_copy(
                    row[:, 2 * n_pairs:2 * n_pairs + n_act],
                    ps_row[:, 2 * np_cols:2 * np_cols + n_act])
            if te_idx:
                nc.vector.tensor_copy(row[:, 2 * n_pairs + n_act:NB],
                                      ps_ter[:, 0:len(te_idx)])

            # AllReduce [1, 32] across cores via DRAM bounce
            cc_in = dram_pool.tile([1, 32], dt.float32, name="cc_in")
            cc_out = dram_pool.tile([1, 32], dt.float32, name="cc_out")
            nc.sync.dma_start(out=cc_in[:], in_=row[:])
            nc.gpsimd.collective_compute(
                "AllReduce", Alu.add,
                replica_groups=[list(range(NUM_CORES))],
                ins=[cc_in.opt()], outs=[cc_out.opt()],
            )
            g = fin_pool.tile([1, 32], dt.float32, name="g_row")
            nc.sync.dma_start(out=g[:], in_=cc_out[:])
            nc.sync.dma_start(out=cnt_ext[:], in_=g[:])

            # ---- AP formula on partition 0 ----
            fp_c = g[:, 0:T]
            tp_c = g[:, T:2 * T]
            P_c = g[:, 2 * T:2 * T + 1]

            num = fin_pool.tile([1, T], dt.float32, name="num")  # tp+eps
            nc.vector.tensor_scalar(out=num[:], in0=tp_c, scalar1=EPS,
                                    scalar2=None, op0=Alu.add)
            den = fin_pool.tile([1, T], dt.float32, name="den")  # tp+fp+eps
            nc.vector.scalar_tensor_tensor(out=den[:], in0=tp_c, scalar=EPS,
                                           in1=fp_c, op0=Alu.add, op1=Alu.add)
            rden = fin_pool.tile([1, T], dt.float32, name="rden")
            nc.vector.reciprocal(out=rden[:], in_=den[:])
            prec = fin_pool.tile([1, T], dt.float32, name="prec")
            nc.vector.tensor_tensor(out=prec[:], in0=num[:], in1=rden[:],
                                    op=Alu.mult)
            denr = fin_pool.tile([1, 1], dt.float32, name="denr")
            nc.vector.tensor_scalar(out=denr[:], in0=P_c, scalar1=EPS,
                                    scalar2=None, op0=Alu.add)
            rdenr = fin_pool.tile([1, 1], dt.float32, name="rdenr")
            nc.vector.reciprocal(out=rdenr[:], in_=denr[:])
            rec = fin_pool.tile([1, T], dt.float32, name="rec")
            nc.vector.tensor_scalar(out=rec[:], in0=num[:], scalar1=rdenr[:],
                                    scalar2=None, op0=Alu.mult)

            p_row = fin_pool.tile([1, T + 2], dt.float32, name="p_row")
            r_row = fin_pool.tile([1, T + 2], dt.float32, name="r_row")
            nc.vector.memset(p_row[:], 0.0)
            nc.vector.memset(r_row[:], 0.0)
            nc.vector.tensor_copy(p_row[:, 1:T + 1], prec[:])
            nc.vector.memset(p_row[:, T + 1:T + 2], 1.0)
            nc.vector.tensor_copy(r_row[:, 1:T + 1], rec[:])
            nc.vector.memset(r_row[:, 0:1], 1.0)

            dr = fin_pool.tile([1, T + 1], dt.float32, name="dr")
            nc.vector.tensor_tensor(out=dr[:], in0=r_row[:, 1:T + 2],
                                    in1=r_row[:, 0:T + 1], op=Alu.subtract)
            ps_p = fin_pool.tile([1, T + 1], dt.float32, name="ps_p")
            nc.vector.tensor_tensor(out=ps_p[:], in0=p_row[:, 1:T + 2],
                                    in1=p_row[:, 0:T + 1], op=Alu.add)
            prod = fin_pool.tile([1, T + 1], dt.float32, name="prod")
            nc.vector.tensor_tensor(out=prod[:], in0=dr[:], in1=ps_p[:],
                                    op=Alu.mult)
            area2 = fin_pool.tile([1, 1], dt.float32, name="area2")
            nc.vector.tensor_reduce(out=area2[:], in_=prod[:],
                                    axis=mybir.AxisListType.X, op=Alu.add)
            # recall decreases in t -> signed area <= 0; |area| = -0.5*sum
            res = fin_pool.tile([1, 1], dt.float32, name="res")
            nc.vector.tensor_scalar(out=res[:], in0=area2[:], scalar1=-0.5,
                                    scalar2=None, op0=Alu.mult)
            nc.sync.dma_start(out=out_ext[:], in_=res[:])

    nc.compile()
    return nc, len(te_idx)


def _host_consts(n_te):
    # one-hot column matrices for the TE lane: oh[:, i*32 + i2] layout —
    # block i is a [128, n_te] matrix whose column i is ones.
    nte = max(1, n_te)
    oh = np.zeros((P, nte * 32), dtype=np.float32)
    for i in range(n_te):
        oh[:, i * 32 + i] = 1.0
    from ml_dtypes import bfloat16

    oh = oh.astype(bfloat16)
    ident = np.eye(32, dtype=np.float32)
    return oh, ident


def _prepare_inputs(y_pred, y_true, n_te):
    s = np.ascontiguousarray(np.asarray(y_pred)[:, 1]).reshape(-1)
    s = s.astype(np.float32, copy=False)
    y = np.asarray(y_true).reshape(-1).astype(np.int32, copy=False)
    n = s.size
    assert n == NUM_CORES * P * FTOT, n
    s_sh = s.reshape(NUM_CORES, P, FTOT)
    y_sh = y.reshape(NUM_CORES, P, FTOT)
    oh, ident = _host_consts(n_te)
    return [
        {"s": s_sh[i], "y": y_sh[i], "oh": oh, "ident": ident}
        for i in range(NUM_CORES)
    ]


def _run(y_pred, y_true, thresholds, trace=False):
    from concourse.bass_utils import run_bass_kernel_spmd

    nc, n_te = _build(thresholds)
    in_maps = _prepare_inputs(y_pred, y_true, n_te)
    res = run_bass_kernel_spmd(
        nc, in_maps, core_ids=list(range(NUM_CORES)), trace=trace
    )
    out = np.asarray(res.results[0]["out"], np.float32).reshape(())
    return out, res


def kernel(y_pred, y_true, thresholds):
    out, _ = _run(y_pred, y_true, thresholds, trace=False)
    return out
